# revision 1
# baseline (speedup 1.0000x reference)
"""GRU (ragged sequences) Trainium2 Bass kernel — chunked-Picard formulation.

Instead of a latency-bound sequential scan (one dependency chain of
PE->Act->DVE->Act->DVE per timestep, ~2.2us/tick on TRN2), the GRU is
solved per time-chunk by Picard iteration, which converges in a handful
of sweeps because the GRU step map is strongly contractive here:

  repeat M times (per chunk of K steps, per sequence):
    gates from the PREVIOUS iterate's trajectory (wide, parallel over t):
      s_rz  = gi_rz + W_rz h_prev[t-1]          (PE matmuls, f32r)
      r, z  = sigmoid(s_rz)                      (one wide Act op)
      pre   = gi_n + r * (W_n h_prev[t-1] + bhn) (stt + PE psum-accumulate)
      n     = tanh(pre)
    then the h-recurrence is AFFINE diagonal: h_t = z_t h_{t-1} + (1-z_t) n_t
    solved exactly along the chunk by ONE tensor_tensor_scan instruction.

  The variable-length mask folds into gi_z as +1e4 (z=1 -> h frozen), which
  also reproduces the reference's frozen outputs past seq_len.

Sequences are sorted by length and interleaved across cores (core c gets
ranks c, c+8, ...), so all cores share one live pattern; chunks whose
sequence group is entirely finished are skipped (frozen h is broadcast to
the output instead). x is host-pretransposed to [B, I, T] so gi needs no
on-device transpose; output is [B, H, T], host-retransposed.
"""

import sys
import numpy as np

sys.path.insert(0, "/opt/trn_rl_repo")

B, T_FULL, I, H = 64, 2048, 128, 128
NCORES = 8
BC = B // NCORES          # sequences per core
M_SWEEPS = 4
KMAX = 512

_CACHE = {}


def _plan_chunks(T):
    """Chunk plan: 512-wide for the first half, 256-wide tail."""
    if T <= KMAX:
        return [(0, T)]
    return [(t, KMAX) for t in range(0, T, KMAX)]


def _assignment(seq_len, T):
    """Interleaved sorted assignment: core c, slot p <- rank p*NCORES + c.
    Returns perm [BC, NCORES] of original indices and live[p][chunk]."""
    order = np.argsort(-np.asarray(seq_len), kind="stable")
    perm = order.reshape(BC, NCORES)           # [slot, core]
    plan = _plan_chunks(T)
    live = []
    for p in range(BC):
        maxlen = int(np.asarray(seq_len)[perm[p]].max())
        live.append(tuple(maxlen > t0 for t0, _ in plan))
    return perm, tuple(live)


def _build(T, live):
    from contextlib import ExitStack
    import concourse.bacc as bacc
    import concourse.mybir as mybir
    import concourse.tile as tile

    plan = _plan_chunks(T)
    KM = min(KMAX, T)

    f32 = mybir.dt.float32
    f32r = mybir.dt.float32r
    Alu = mybir.AluOpType
    Act = mybir.ActivationFunctionType

    nc = bacc.Bacc("TRN2", target_bir_lowering=False, debug=False,
                   num_devices=NCORES)

    xt = nc.dram_tensor("xt", [BC, I, T], f32r, kind="ExternalInput").ap()
    wih3 = nc.dram_tensor("wih3", [I, 3 * H], f32r, kind="ExternalInput").ap()
    whh3 = nc.dram_tensor("whh3", [H, 3 * H], f32r, kind="ExternalInput").ap()
    gibt = nc.dram_tensor("gibt", [3, 128], f32r, kind="ExternalInput").ap()
    bhn = nc.dram_tensor("bhn", [H, 1], f32, kind="ExternalInput").ap()
    mrow = nc.dram_tensor("mrow", [1, BC * T], f32r, kind="ExternalInput").ap()
    ident = nc.dram_tensor("ident", [128, 128], f32r, kind="ExternalInput").ap()
    onesd = nc.dram_tensor("onesd", [1, 512], f32r, kind="ExternalInput").ap()
    yt = nc.dram_tensor("yt", [BC, H, T], f32r, kind="ExternalOutput").ap()

    with tile.TileContext(nc) as tc, ExitStack() as ctx:
        const = ctx.enter_context(tc.tile_pool(name="const", bufs=1))
        xpool = ctx.enter_context(tc.tile_pool(name="x", bufs=3))
        gipool = ctx.enter_context(tc.tile_pool(name="gi", bufs=2))
        hppool = ctx.enter_context(tc.tile_pool(name="hp", bufs=3))
        epool = ctx.enter_context(tc.tile_pool(name="entry", bufs=3))
        wpool = ctx.enter_context(tc.tile_pool(name="work", bufs=4))
        ps_rz = ctx.enter_context(tc.tile_pool(name="ps_rz", bufs=3, space="PSUM"))
        ps_gi = ctx.enter_context(tc.tile_pool(name="ps_gi", bufs=2, space="PSUM"))

        wih_sb = const.tile([128, 3 * H], f32r, tag="wih")
        nc.sync.dma_start(out=wih_sb[:], in_=wih3)
        whh_sb = const.tile([128, 3 * H], f32r, tag="whh")
        nc.sync.dma_start(out=whh_sb[:], in_=whh3)
        gib_rows = []
        for g in range(3):
            row = const.tile([1, 128], f32r, tag=f"gib{g}")
            nc.sync.dma_start(out=row[:], in_=gibt[g:g + 1, :])
            gib_rows.append(row)
        bhn_sb = const.tile([128, 1], f32, tag="bhn")
        nc.sync.dma_start(out=bhn_sb[:], in_=bhn)
        id_sb = const.tile([128, 128], f32r, tag="id")
        nc.sync.dma_start(out=id_sb[:], in_=ident)
        ones_sb = const.tile([1, KM], f32r, tag="ones")
        nc.sync.dma_start(out=ones_sb[:], in_=onesd[0:1, 0:KM])
        zero_e = const.tile([128, 1], f32, tag="zeroe")
        nc.vector.memset(zero_e[:], 0.0)
        brc_sb = const.tile([128, KM], f32, tag="brc")
        nc.vector.memset(brc_sb[:], 0.0)

        entry = {b: zero_e for b in range(BC)}

        for c, (t0, K) in enumerate(plan):
            gis = {}
            hps = {}
            # ---- chunk preamble: gi for live sequences ------------------
            def preamble(b):
                xtile = xpool.tile([128, KM], f32r, tag="x")
                nc.sync.dma_start(out=xtile[:, 0:K], in_=xt[b, :, t0:t0 + K])
                mtile = xpool.tile([1, KM], f32r, tag="m")
                nc.sync.dma_start(out=mtile[0:1, 0:K],
                                  in_=mrow[0:1, b * T + t0: b * T + t0 + K])
                gi_sb = gipool.tile([128, 3 * KM], f32r, tag=f"gi{b}")
                for g in range(3):
                    pg = ps_gi.tile([128, KM], f32, tag="pgi")
                    nc.tensor.matmul(pg[:, 0:K],
                                     wih_sb[:, g * 128:(g + 1) * 128],
                                     xtile[:, 0:K], start=True, stop=False)
                    nc.tensor.matmul(pg[:, 0:K], gib_rows[g][0:1, :],
                                     ones_sb[0:1, 0:K], start=False,
                                     stop=(g != 1))
                    if g == 1:  # fold +1e4 mask into gi_z
                        nc.tensor.matmul(
                            pg[:, 0:K], ones_sb[0:1, 0:128], mtile[0:1, 0:K],
                            start=False, stop=True)
                    dst = gi_sb[:, g * KM:g * KM + K]
                    if g == 2:
                        nc.vector.tensor_copy(out=dst, in_=pg[:, 0:K])
                    else:
                        nc.scalar.copy(dst, pg[:, 0:K])
                gis[b] = gi_sb
                # hp trajectory tile: col 0 = h_entry, cols 1..K = h_1..h_K
                hp = hppool.tile([128, KM + 1], f32r, tag=f"hp{b}")
                # sweep-1 init guess: h_prev[t] = h_entry for all t.
                # brc_sb as the (zero) shape donor: no false dep on gi copies,
                # so sweep-0's gh matmuls can start during the gi preamble.
                nc.gpsimd.tensor_scalar(out=hp[:, 0:K], in0=brc_sb[:, 0:K],
                                        scalar1=0.0, scalar2=entry[b][:, 0:1],
                                        op0=Alu.mult, op1=Alu.add)
                hps[b] = hp

            # ---- Picard sweeps ------------------------------------------
            def sweep(b):
                hp, gi_sb = hps[b], gis[b]
                # psum regions pinned to 512-col bank boundaries: the
                # start=False accumulate below relies on s_z's group
                # owning its bank exclusively.
                prz = ps_rz.tile([128, 2 * KMAX], f32, tag="rz")
                Z0 = KMAX
                for g in range(2):
                    dst = prz[:, g * KMAX:g * KMAX + K]
                    nc.tensor.matmul(dst,
                                     whh_sb[:, g * 128:(g + 1) * 128],
                                     hp[:, 0:K], start=True, stop=False)
                    nc.tensor.matmul(dst, id_sb[:],
                                     gi_sb[:, g * KM:g * KM + K],
                                     start=False, stop=True)
                rz = wpool.tile([128, 2 * KM], f32, tag="rz_sb")
                prz3 = prz.rearrange("p (g k) -> p g k", g=2)
                rz3 = rz.rearrange("p (g k) -> p g k", g=2)
                nc.scalar.activation(rz3[:, :, 0:K], prz3[:, :, 0:K],
                                     Act.Sigmoid)
                # ghn reuses the dead s_r bank (freed by the sigmoid read)
                nc.tensor.matmul(prz[:, 0:K], whh_sb[:, 256:384],
                                 hp[:, 0:K], start=True, stop=True,
                                 skip_group_check=True)
                # t1 = (ghn + bhn) * r   -> overwrite dead s_z psum region
                nc.vector.scalar_tensor_tensor(
                    out=prz[:, Z0:Z0 + K], in0=prz[:, 0:K],
                    scalar=bhn_sb[:, 0:1],
                    in1=rz[:, 0:K], op0=Alu.add, op1=Alu.mult)
                # pre = t1 + gi_n: PE accumulates gi_n onto t1 in-place.
                # The psum has_written bits set by the s_z matmuls above
                # survive the DVE overwrite, so start=False adds.
                nc.tensor.matmul(prz[:, Z0:Z0 + K], id_sb[:],
                                 gi_sb[:, 2 * KM:2 * KM + K],
                                 start=False, stop=True,
                                 skip_group_check=True)
                # nneg = tanh(-pre) = -n  (free negation via scale)
                n_sb = wpool.tile([128, KM], f32, tag="n_sb")
                nc.scalar.activation(n_sb[:, 0:K], prz[:, Z0:Z0 + K],
                                     Act.Tanh, scale=-1.0)
                # un = (z-1)*(-n) = (1-z)*n  in one DVE op
                un_sb = wpool.tile([128, KM], f32, tag="un_sb")
                nc.vector.scalar_tensor_tensor(
                    out=un_sb[:, 0:K], in0=rz[:, K:2 * K], scalar=1.0,
                    in1=n_sb[:, 0:K], op0=Alu.subtract, op1=Alu.mult)
                # exact affine solve along the chunk:
                # h_t = z_t h_{t-1} + un_t
                nc.vector.tensor_tensor_scan(
                    out=hp[:, 1:K + 1], data0=rz[:, K:2 * K],
                    data1=un_sb[:, 0:K], initial=entry[b][:, 0:1],
                    op0=Alu.mult, op1=Alu.add)

            def finish(b):
                if live[b][c]:
                    hp = hps[b]
                    e_next = epool.tile([128, 1], f32, tag=f"e{b}")
                    nc.gpsimd.tensor_copy(out=e_next[:], in_=hp[:, K:K + 1])
                    entry[b] = e_next
                    nc.sync.dma_start(out=yt[b, :, t0:t0 + K],
                                      in_=hp[:, 1:K + 1])
                else:
                    # sequence group finished: output = frozen h, broadcast
                    fz = wpool.tile([128, KM], f32r, tag="fz", bufs=2)
                    nc.gpsimd.tensor_scalar(out=fz[:, 0:K],
                                            in0=brc_sb[:, 0:K],
                                            scalar1=0.0,
                                            scalar2=entry[b][:, 0:1],
                                            op0=Alu.mult, op1=Alu.add)
                    nc.sync.dma_start(out=yt[b, :, t0:t0 + K],
                                      in_=fz[:, 0:K])

            # sweep 0 issued right after each slot's preamble (short engine
            # queues at chunk start); epilogue folded into the last sweep so
            # the next chunk's preamble unblocks per-slot.
            for b in range(BC):
                if live[b][c]:
                    preamble(b)
            for b in range(BC):
                if live[b][c]:
                    sweep(b)
            for s in range(1, M_SWEEPS):
                for b in range(BC):
                    if not live[b][c]:
                        continue
                    sweep(b)
                    if s == M_SWEEPS - 1:
                        finish(b)
            for b in range(BC):
                if not live[b][c]:
                    finish(b)

    nc.compile()
    return nc


def _host_prep(x, seq_len, w_ih, w_hh, b_ih, b_hh, perm):
    T = x.shape[1]
    x = np.asarray(x, np.float32)
    xt_all = np.ascontiguousarray(x.transpose(0, 2, 1))  # [B, I, T]
    wih3 = np.ascontiguousarray(w_ih.T.astype(np.float32))
    whh3 = np.ascontiguousarray(w_hh.T.astype(np.float32))
    gibt = np.stack([
        (b_ih[0:128] + b_hh[0:128]),
        (b_ih[128:256] + b_hh[128:256]),
        b_ih[256:384],
    ], axis=0).astype(np.float32)
    bhn = b_hh[256:384].astype(np.float32)[:, None]
    identity = np.eye(128, dtype=np.float32)
    seq_len = np.asarray(seq_len).astype(np.int64)
    in_maps = []
    for c in range(NCORES):
        idx = perm[:, c]                       # slot p -> original seq index
        sl = seq_len[idx]
        mask = (np.arange(T)[None, :] >= sl[:, None]).astype(np.float32)
        mrow = (1e4 * mask).reshape(1, BC * T)
        in_maps.append({
            "xt": np.ascontiguousarray(xt_all[idx]),
            "wih3": wih3, "whh3": whh3, "gibt": gibt, "bhn": bhn,
            "mrow": mrow, "ident": identity,
            "onesd": np.ones((1, 512), np.float32),
        })
    return in_maps


LAST_RESULTS = None


def kernel(x, seq_len, w_ih, w_hh, b_ih, b_hh):
    global LAST_RESULTS
    from concourse import bass_utils
    T = x.shape[1]
    perm, live = _assignment(seq_len, T)
    key = (T, live)
    if key not in _CACHE:
        _CACHE[key] = _build(T, live)
    nc = _CACHE[key]
    in_maps = _host_prep(np.asarray(x), np.asarray(seq_len), np.asarray(w_ih),
                         np.asarray(w_hh), np.asarray(b_ih), np.asarray(b_hh),
                         perm)
    res = bass_utils.run_bass_kernel_spmd(nc, in_maps,
                                          core_ids=list(range(NCORES)))
    LAST_RESULTS = res
    B_ = perm.size
    y = np.empty((B_, T, 128), np.float32)
    for c in range(NCORES):
        y[perm[:, c]] = res.results[c]["yt"].transpose(0, 2, 1)
    return np.ascontiguousarray(y)



# revision 6
# speedup vs baseline: 1.1959x; 1.1959x over previous
"""GRU (ragged sequences) Trainium2 Bass kernel — chunked-Picard v2b.

The GRU is solved per time-chunk by Picard iteration (the step map is
strongly contractive), with the h-recurrence solved exactly along the
chunk by one tensor_tensor_scan per sweep:

  sweep s (gates from the previous iterate's trajectory, wide over t):
    s_g  = W_g_hh h_prev[t-1] + W_g_ih x_t + b_g    (PE, f32r, psum accum)
    r, z = sigmoid(s_rz)                            (Act)
    pre  = s_n_ih + r * (W_n_hh h_prev[t-1] + bhn)  (DVE stt + PE accum)
    n    = tanh(pre)                                (Act)
    h_t  = z_t h_{t-1} + (1-z_t) n_t                (exact affine scan, DVE)

v2b vs v1:
  * gi is RECOMPUTED on PE each sweep (Wih x accumulated into the same
    psum group as Whh h) instead of precomputed + evacuated to SBUF:
    kills all three PSUM->SBUF evacuation ops per chunk (DVE was the
    bottleneck engine) at the cost of PE matmuls (PE has headroom).
  * Sweep schedule (rzn, zn, rzn, zn): the r gate is only recomputed on
    sweeps 0 and 2 (rel err 9.9e-3 vs 7.9e-3 for full, budget 2e-2).
  * Variable-width chunk plans per slot: the last chunk of each slot is
    trimmed to the slot's max sequence length (rounded up to 64, min 256
    to keep f32r matmuls at 1 cycle/row): 23 -> 20.5 chunk-equivalents.
  * Ragged masking via host-side x poisoning: for t >= seq_len, x[:,t]
    is replaced by v solving W_z_ih v + b_ih_z = 40, so z saturates to
    exactly 1.0 in fp32 and h freezes bit-exactly.  Kills the mask row
    DMA and the per-chunk mask matmul.
  * Output tail (t >= slot plan end) filled on host from the last column
    instead of on-device broadcast+DMA.

Sequences are sorted by length and interleaved across cores (core c gets
ranks c, c+8, ...) so all cores share one live pattern / one program.
x is host-pretransposed to [B, I, T]; output is [B, H, T].
"""

import sys
import numpy as np

sys.path.insert(0, "/opt/trn_rl_repo")

B, T_FULL, I, H = 64, 2048, 128, 128
NCORES = 8
BC = B // NCORES          # sequences per core
KMAX = 512
SCHED = ("rzn", "zn", "rzn", "zn")

_CACHE = {}


def _plan_slot(maxlen, T):
    """Chunks of 512 plus a trimmed tail in [256, 512] rounded up to 64."""
    plan = []
    t0 = 0
    while t0 + KMAX <= maxlen:
        plan.append((t0, KMAX))
        t0 += KMAX
    rem = maxlen - t0
    if rem > 0:
        w = min(KMAX, max(256, -(-rem // 64) * 64))
        w = min(w, T - t0)
        plan.append((t0, w))
    return tuple(plan)


def _assignment(seq_len, T):
    """Interleaved sorted assignment: core c, slot p <- rank p*NCORES + c."""
    sl = np.asarray(seq_len)
    order = np.argsort(-sl, kind="stable")
    perm = order.reshape(BC, NCORES)           # [slot, core]
    plans = tuple(_plan_slot(int(sl[perm[p]].max()), T) for p in range(BC))
    return perm, plans


def _build(T, plans):
    from contextlib import ExitStack
    import concourse.bacc as bacc
    import concourse.mybir as mybir
    import concourse.tile as tile

    f32 = mybir.dt.float32
    f32r = mybir.dt.float32r
    Alu = mybir.AluOpType
    Act = mybir.ActivationFunctionType

    nrounds = max(len(p) for p in plans)

    nc = bacc.Bacc("TRN2", target_bir_lowering=False, debug=False,
                   num_devices=NCORES)

    xt = nc.dram_tensor("xt", [BC, I, T], f32r, kind="ExternalInput").ap()
    wih3 = nc.dram_tensor("wih3", [I, 3 * H], f32r, kind="ExternalInput").ap()
    whh3 = nc.dram_tensor("whh3", [H, 3 * H], f32r, kind="ExternalInput").ap()
    # per-gate total biases as 1-row weights: r,z: b_ih+b_hh, n: b_ih only
    gibt = nc.dram_tensor("gibt", [3, 128], f32r, kind="ExternalInput").ap()
    bhn = nc.dram_tensor("bhn", [H, 1], f32, kind="ExternalInput").ap()
    onesd = nc.dram_tensor("onesd", [1, KMAX], f32r, kind="ExternalInput").ap()
    yt = nc.dram_tensor("yt", [BC, H, T], f32r, kind="ExternalOutput").ap()

    with tile.TileContext(nc) as tc, ExitStack() as ctx:
        const = ctx.enter_context(tc.tile_pool(name="const", bufs=1))
        xpool = ctx.enter_context(tc.tile_pool(name="x", bufs=2))
        hppool = ctx.enter_context(tc.tile_pool(name="hp", bufs=2))
        rzpool = ctx.enter_context(tc.tile_pool(name="rz", bufs=1))
        npool = ctx.enter_context(tc.tile_pool(name="nn", bufs=8))
        unpool = ctx.enter_context(tc.tile_pool(name="un", bufs=8))
        ps_rz = ctx.enter_context(tc.tile_pool(name="ps_rz", bufs=4, space="PSUM"))

        wih_sb = const.tile([128, 3 * H], f32r, tag="wih")
        nc.sync.dma_start(out=wih_sb[:], in_=wih3)
        whh_sb = const.tile([128, 3 * H], f32r, tag="whh")
        nc.sync.dma_start(out=whh_sb[:], in_=whh3)
        gib_rows = []
        for g in range(3):
            row = const.tile([1, 128], f32r, tag=f"gib{g}", name=f"gib{g}")
            nc.sync.dma_start(out=row[:], in_=gibt[g:g + 1, :])
            gib_rows.append(row)
        bhn_sb = const.tile([128, 1], f32, tag="bhn")
        nc.sync.dma_start(out=bhn_sb[:], in_=bhn)
        ones_sb = const.tile([1, KMAX], f32r, tag="ones")
        nc.sync.dma_start(out=ones_sb[:], in_=onesd)
        zero_e = const.tile([128, 1], f32, tag="zeroe")
        nc.vector.memset(zero_e[:], 0.0)
        brc_sb = const.tile([128, KMAX], f32, tag="brc")
        nc.vector.memset(brc_sb[:], 0.0)

        entry = {b: zero_e[:, 0:1] for b in range(BC)}
        hps, xs, rzs = {}, {}, {}

        def preamble(b, t0, K):
            xtile = xpool.tile([128, KMAX], f32r, tag=f"x{b}", name=f"x{b}")
            nc.sync.dma_start(out=xtile[:, 0:K], in_=xt[b, :, t0:t0 + K])
            xs[b] = xtile
            # hp trajectory tile: col 0 = h_entry, cols 1..K = h_1..h_K.
            # sweep-0 guess: h_prev[t] = h_entry for all t (brc as zero
            # shape-donor: no false dep on anything).
            hp = hppool.tile([128, KMAX + 1], f32r, tag=f"hp{b}", name=f"hp{b}")
            nc.gpsimd.tensor_scalar(out=hp[:, 0:K], in0=brc_sb[:, 0:K],
                                    scalar1=0.0, scalar2=entry[b],
                                    op0=Alu.mult, op1=Alu.add)
            hps[b] = hp
            rzs[b] = rzpool.tile([128, 2 * KMAX], f32, tag=f"rz{b}",
                                 name=f"rz{b}")

        def gate_psum(dst, g, hp, xtile, K):
            """dst = W_g_hh h + W_g_ih x + b_g  (3-matmul psum group)."""
            nc.tensor.matmul(dst, whh_sb[:, g * 128:(g + 1) * 128],
                             hp[:, 0:K], start=True, stop=False)
            nc.tensor.matmul(dst, wih_sb[:, g * 128:(g + 1) * 128],
                             xtile[:, 0:K], start=False, stop=False)
            nc.tensor.matmul(dst, gib_rows[g][0:1, :], ones_sb[0:1, 0:K],
                             start=False, stop=True)

        def sweep(b, t0, K, gates):
            hp, xtile, rz = hps[b], xs[b], rzs[b]
            # psum pinned to 512-col bank boundaries; the start=False
            # accumulates rely on exclusive bank ownership.
            prz = ps_rz.tile([128, 2 * KMAX], f32, tag="przn")
            Z0 = KMAX
            if "r" in gates:
                gate_psum(prz[:, 0:K], 0, hp, xtile, K)
            gate_psum(prz[:, Z0:Z0 + K], 1, hp, xtile, K)
            if "r" in gates:
                prz3 = prz.rearrange("p (g k) -> p g k", g=2)
                rz3 = rz.rearrange("p (g k) -> p g k", g=2)
                nc.scalar.activation(rz3[:, :, 0:K], prz3[:, :, 0:K],
                                     Act.Sigmoid)
            else:
                nc.scalar.activation(rz[:, Z0:Z0 + K], prz[:, Z0:Z0 + K],
                                     Act.Sigmoid)
            # ghn into the (dead or unused) r psum region
            nc.tensor.matmul(prz[:, 0:K], whh_sb[:, 256:384], hp[:, 0:K],
                             start=True, stop=True, skip_group_check=True)
            # t1 = (ghn + bhn) * r  -> overwrite dead s_z psum region
            nc.vector.scalar_tensor_tensor(
                out=prz[:, Z0:Z0 + K], in0=prz[:, 0:K],
                scalar=bhn_sb[:, 0:1],
                in1=rz[:, 0:K], op0=Alu.add, op1=Alu.mult)
            # pre = t1 + (W_n_ih x + b_ih_n): PE accumulates onto t1
            # in-place (psum has_written bits from s_z matmuls survive the
            # DVE overwrite, so start=False adds).
            nc.tensor.matmul(prz[:, Z0:Z0 + K], wih_sb[:, 256:384],
                             xtile[:, 0:K], start=False, stop=False,
                             skip_group_check=True)
            nc.tensor.matmul(prz[:, Z0:Z0 + K], gib_rows[2][0:1, :],
                             ones_sb[0:1, 0:K], start=False, stop=True,
                             skip_group_check=True)
            # nneg = tanh(-pre) = -n  (free negation via scale)
            nsb = npool.tile([128, KMAX], f32, tag="nn", name="nsb")
            nc.scalar.activation(nsb[:, 0:K], prz[:, Z0:Z0 + K],
                                 Act.Tanh, scale=-1.0)
            # un = (z-1)*(-n) = (1-z)*n
            un = unpool.tile([128, KMAX], f32, tag="un", name="un")
            nc.vector.scalar_tensor_tensor(
                out=un[:, 0:K], in0=rz[:, Z0:Z0 + K], scalar=1.0,
                in1=nsb[:, 0:K], op0=Alu.subtract, op1=Alu.mult)
            # exact affine solve along the chunk: h_t = z_t h_{t-1} + un_t
            nc.vector.tensor_tensor_scan(
                out=hp[:, 1:K + 1], data0=rz[:, Z0:Z0 + K],
                data1=un[:, 0:K], initial=entry[b],
                op0=Alu.mult, op1=Alu.add)

        def finish(b, t0, K):
            hp = hps[b]
            nc.sync.dma_start(out=yt[b, :, t0:t0 + K], in_=hp[:, 1:K + 1])
            entry[b] = hp[:, K:K + 1].bitcast(f32)

        for ci in range(nrounds):
            livebs = [b for b in range(BC) if len(plans[b]) > ci]
            for b in livebs:
                t0, K = plans[b][ci]
                preamble(b, t0, K)
            for s, gates in enumerate(SCHED):
                for b in livebs:
                    t0, K = plans[b][ci]
                    sweep(b, t0, K, gates)
                    if s == len(SCHED) - 1:
                        finish(b, t0, K)

    nc.compile()
    return nc


def _host_prep(x, seq_len, w_ih, w_hh, b_ih, b_hh, perm):
    T = x.shape[1]
    x = np.asarray(x, np.float32)
    w_ih = np.asarray(w_ih, np.float32)
    w_hh = np.asarray(w_hh, np.float32)
    b_ih = np.asarray(b_ih, np.float32)
    b_hh = np.asarray(b_hh, np.float32)
    seq_len = np.asarray(seq_len).astype(np.int64)
    xt_all = np.ascontiguousarray(x.transpose(0, 2, 1))  # [B, I, T]
    # Poison columns t >= seq_len so that gi_z + b_ih_z ~= 60: z saturates
    # to exactly 1.0 in fp32 (gh_z is bounded by ~6) and h freezes
    # bit-exactly, reproducing the reference's frozen outputs past seq_len.
    # Truncated-SVD solve: tiny singular directions of W_z_ih are dropped so
    # that ||v|| stays small enough for the PE's reduced-precision f32r
    # accumulation (a full solve can give ||v|| ~ 1e6 on an ill-conditioned
    # W_z and f32r noise ~1e3 destroys the freeze).  Dropping sigma_i only
    # perturbs s_z by ~ +-c|u_i^T 1||u_i| << c, still far above saturation.
    Wz = w_ih[H:2 * H].astype(np.float64)
    c = np.full(H, 60.0) - b_ih[H:2 * H].astype(np.float64)
    U, S, Vt = np.linalg.svd(Wz)
    Sinv = np.where(S >= S.max() / 300.0, 1.0 / S, 0.0)
    v = (Vt.T @ (Sinv * (U.T @ c))).astype(np.float32)
    for b in range(B):
        if seq_len[b] < T:
            xt_all[b, :, seq_len[b]:] = v[:, None]
    wih3 = np.ascontiguousarray(w_ih.T)
    whh3 = np.ascontiguousarray(w_hh.T)
    gibt = np.stack([
        b_ih[0:H] + b_hh[0:H],
        b_ih[H:2 * H] + b_hh[H:2 * H],
        b_ih[2 * H:],
    ], axis=0).astype(np.float32)
    bhn_v = b_hh[2 * H:].astype(np.float32)[:, None]
    in_maps = []
    for c in range(NCORES):
        idx = perm[:, c]                       # slot p -> original seq index
        in_maps.append({
            "xt": np.ascontiguousarray(xt_all[idx]),
            "wih3": wih3, "whh3": whh3, "gibt": gibt, "bhn": bhn_v,
            "onesd": np.ones((1, KMAX), np.float32),
        })
    return in_maps


LAST_RESULTS = None


def kernel(x, seq_len, w_ih, w_hh, b_ih, b_hh):
    global LAST_RESULTS
    from concourse import bass_utils
    T = x.shape[1]
    perm, plans = _assignment(seq_len, T)
    key = (T, plans)
    if key not in _CACHE:
        _CACHE[key] = _build(T, plans)
    nc = _CACHE[key]
    in_maps = _host_prep(np.asarray(x), np.asarray(seq_len), np.asarray(w_ih),
                         np.asarray(w_hh), np.asarray(b_ih), np.asarray(b_hh),
                         perm)
    res = bass_utils.run_bass_kernel_spmd(nc, in_maps,
                                          core_ids=list(range(NCORES)))
    LAST_RESULTS = res
    y = np.empty((B, T, H), np.float32)
    for c in range(NCORES):
        ytc = np.array(res.results[c]["yt"])   # [BC, H, T]
        for p in range(BC):
            t0, K = plans[p][-1]
            t_end = t0 + K
            if t_end < T:
                # past the slot's plan end, h is frozen: replicate last col
                ytc[p, :, t_end:] = ytc[p, :, t_end - 1][:, None]
        y[perm[:, c]] = ytc.transpose(0, 2, 1)
    return np.ascontiguousarray(y)
